# revision 15
# baseline (speedup 1.0000x reference)
"""Multi-head attention (B=2, N=2048, d_model=1024, H=16) on 8 NeuronCores.

Sharding: data-parallel on batch (2) x tensor-parallel on heads (4 groups of
4 heads). Core c handles batch c//4, head-group c%4; the host sums the 4
output-projection partials per batch.

Precision plan (gate is relmax 2e-2; attention here averages ~uniformly over
~10^3 keys, so the output is CLT-small and independent per-key quantization
noise does NOT average away relative to it — plain fp8 anywhere in the
v->output chain costs ~2-3e-2 alone):
  - Projections run as residual-fp8: the host ships x and 32*wq as an fp8
    value plus an fp8 residual; three DoubleRow passes (x8*w8 + xr8*w8 +
    x8*wr8) give ~bf16 accuracy at 0.75x the bf16 matmul cost.
  - q/k are STORED fp8 (the one affordable lossy point, ~1.2e-2): score
    matmuls then run DoubleRow at 0.5 cycles/row by pairing the real K-tile
    with a zero slot (stride-0 {k,k} stationary x {q,0} moving).
  - P, V, attention output, and the output projection stay bf16.

Softmax skips max-subtraction (scores bounded ~+-3): exp runs on ACT only
(~73us), second to the PE (~88us bottleneck). DVE does the PSUM->SBUF moves
(bias adds, normalization, output copies); GPSIMD does the causal mask
multiplies and the denominator broadcasts.
"""

import sys

if "/opt/trn_rl_repo" not in sys.path:
    sys.path.insert(0, "/opt/trn_rl_repo")

import numpy as np
import ml_dtypes

import concourse.bass as bass
import concourse.mybir as mybir
import concourse.tile as tile
from concourse import bacc
from concourse.bass_utils import run_bass_kernel_spmd

B, N, D, H = 2, 2048, 1024, 16
DV = D // H  # 64
HPC = H // 4  # heads per core: 4
DHC = HPC * DV  # head dims per core: 256
DT = D // 128  # 8 din-tiles
F8 = mybir.dt.float8e4
BF = mybir.dt.bfloat16
F32 = mybir.dt.float32
F16 = mybir.dt.float16
EXP = mybir.ActivationFunctionType.Exp
DR = mybir.MatmulPerfMode.DoubleRow
NPF8 = ml_dtypes.float8_e4m3
NPBF = ml_dtypes.bfloat16
SQ = 32.0  # wq pre-scale (q/k/v live at 32x true value on chip)
ESCALE = 0.125 / (SQ * SQ)  # exp scale: undo q*k scale, apply 1/sqrt(dv)
OSCALE = SQ  # output partial leaves chip at 32x (host divides)

_CACHE = {}


def build_nc():
    nc = bacc.Bacc("TRN2", target_bir_lowering=False, debug=False)
    dins = {}
    for nm in ("xq8", "xqr8", "xk8", "xkr8", "xv8", "xvr8"):
        dins[nm] = nc.dram_tensor(nm, [D, N], F8, kind="ExternalInput")
    wq8_d = nc.dram_tensor("wq8", [D, DHC], F8, kind="ExternalInput")
    wqr8_d = nc.dram_tensor("wqr8", [D, DHC], F8, kind="ExternalInput")
    woT_d = nc.dram_tensor("woT", [DHC, D], BF, kind="ExternalInput")
    bq_d = nc.dram_tensor("bq", [DHC], F32, kind="ExternalInput")
    utm_d = nc.dram_tensor("utm", [128, 128], BF, kind="ExternalInput")
    zz_d = nc.dram_tensor("zz", [128, N], F8, kind="ExternalInput")
    yT_d = nc.dram_tensor("yT", [D, N], F16, kind="ExternalOutput")

    with tile.TileContext(nc) as tc:
        with (
            tc.tile_pool(name="consts", bufs=1) as consts,
            tc.tile_pool(name="xin", bufs=1) as xin,
            tc.tile_pool(name="prod", bufs=1) as prod,
            tc.tile_pool(name="work", bufs=3) as work,
            tc.tile_pool(name="norm", bufs=3) as norm,
            tc.tile_pool(name="yout", bufs=2) as yout,
            tc.tile_pool(name="ps", bufs=1, space="PSUM") as ps,
        ):
            # ---- weights + constants ----
            wq8 = consts.tile([128, DT, DHC], F8, name="wq8")
            nc.sync.dma_start(
                out=wq8, in_=wq8_d.ap().rearrange("(j p) c -> p j c", p=128)
            )
            wqr8 = consts.tile([128, DT, DHC], F8, name="wqr8")
            nc.sync.dma_start(
                out=wqr8, in_=wqr8_d.ap().rearrange("(j p) c -> p j c", p=128)
            )
            bq_pp = consts.tile([128, 2], F32, name="bq_pp")
            nc.sync.dma_start(
                out=bq_pp, in_=bq_d.ap().rearrange("(c p) -> p c", p=128)
            )
            bq_row = consts.tile([1, DHC], F32, name="bq_row")
            nc.sync.dma_start(
                out=bq_row, in_=bq_d.ap().rearrange("(a c) -> a c", a=1)
            )
            bq_bc = consts.tile([128, DHC], F32, name="bq_bc")
            nc.gpsimd.partition_broadcast(bq_bc, bq_row)
            utm = consts.tile([128, 128], BF, name="utm")
            nc.sync.dma_start(out=utm, in_=utm_d.ap())

            # q/k projection outputs (fp8); qT slot 1 is a persistent zero
            # pad for the score DoubleRow trick
            qT = [prod.tile([128, 2, N], F8, name=f"qT{p}") for p in range(2)]
            kT = [prod.tile([128, N], F8, name=f"kT{p}") for p in range(2)]
            for p in range(2):
                nc.sync.dma_start(out=qT[p][:, 1, :], in_=zz_d.ap())
            xaT = prod.tile([128, 2, N], BF, name="xaT")

            # ---- bulk inputs: k first (kT-proj fills the load window) ----
            xk8 = xin.tile([128, DT, N], F8, name="xk8")
            xkr8 = xin.tile([128, DT, N], F8, name="xkr8")
            xq8 = xin.tile([128, DT, N], F8, name="xq8")
            xqr8 = xin.tile([128, DT, N], F8, name="xqr8")
            xv8 = xin.tile([128, DT, N], F8, name="xv8")
            xvr8 = xin.tile([128, DT, N], F8, name="xvr8")

            def load_slice(t, d, n0, n1):
                nc.sync.dma_start(
                    out=t[:, :, n0:n1],
                    in_=d.ap()[:, n0:n1].rearrange("(j p) n -> p j n", p=128),
                )

            def load_j2(t, d, j2):
                nc.sync.dma_start(
                    out=t[:, j2 : j2 + 2, :],
                    in_=d.ap()[j2 * 128 : (j2 + 2) * 128, :].rearrange(
                        "(j p) n -> p j n", p=128
                    ),
                )

            # incremental chunk-wise loads matched to the unit schedule:
            # chunk c of k/q lands just before the (c,*) bursts need it
            def load_chunk(c, tensors):
                for t, d in tensors:
                    load_slice(t, d, c * 512, (c + 1) * 512)

            kq = [(xk8, dins["xk8"]), (xkr8, dins["xkr8"]),
                  (xq8, dins["xq8"]), (xqr8, dins["xqr8"])]
            vv = [(xv8, dins["xv8"]), (xvr8, dins["xvr8"])]
            load_chunk(0, kq)
            load_chunk(0, vv)
            load_chunk(1, kq)
            load_chunk(1, vv)
            load_chunk(2, kq)
            load_chunk(3, kq)
            load_chunk(2, vv)
            load_chunk(3, vv)
            woT = consts.tile([128, 2, D], BF, name="woT")
            nc.sync.dma_start(
                out=woT, in_=woT_d.ap().rearrange("(q p) c -> p q c", p=128)
            )

            # vp[m]: V tile for key-tile m, per head, with a trailing ones
            # column producing the softmax denominator (bf16)
            vp = [
                prod.tile([128, HPC, DV + 1], BF, name=f"vp{m}")
                for m in range(16)
            ]

            def proj_qk(xs, dst, c, p):
                """Residual-fp8 projection: 3 DoubleRow passes over 4
                din-tile pairs, accumulating in one PSUM group."""
                x8, xr8 = xs
                pp = ps.tile([128, 512], F32, name="prj_qk", tag="prj", bufs=2)
                passes = [(wq8, x8), (wq8, xr8), (wqr8, x8)]
                for pi, (w, x) in enumerate(passes):
                    for j2 in range(0, DT, 2):
                        nc.tensor.matmul(
                            pp,
                            w[:, j2 : j2 + 2, p * 128 : (p + 1) * 128],
                            x[:, j2 : j2 + 2, c * 512 : (c + 1) * 512],
                            start=(pi == 0 and j2 == 0),
                            stop=(pi == 2 and j2 == DT - 2),
                            perf_mode=DR,
                        )
                if dst is qT:
                    out = dst[p][:, 0, c * 512 : (c + 1) * 512]
                else:
                    out = dst[p][:, c * 512 : (c + 1) * 512]
                nc.vector.tensor_scalar_add(out, pp, bq_pp[:, p : p + 1])

            def proj_v(m):
                pv = ps.tile([128, 512], F32, name="prj_v", tag="prj", bufs=2)
                pvv = pv[:, 0:DHC]
                passes = [(xv8, wq8), (xvr8, wq8), (xv8, wqr8)]
                for pi, (x, w) in enumerate(passes):
                    for j2 in range(0, DT, 2):
                        nc.tensor.matmul(
                            pvv,
                            x[:, j2 : j2 + 2, m * 128 : (m + 1) * 128],
                            w[:, j2 : j2 + 2, :],
                            start=(pi == 0 and j2 == 0),
                            stop=(pi == 2 and j2 == DT - 2),
                            perf_mode=DR,
                        )
                nc.vector.tensor_add(
                    vp[m][:, :, 0:DV],
                    pvv.rearrange("p (h d) -> p h d", h=HPC),
                    bq_bc.rearrange("p (h d) -> p h d", h=HPC),
                )
                nc.vector.memset(vp[m][:, :, DV : DV + 1], 1.0)

            def outproj_t(c, t, act_copy=False):
                yp = ps.tile([128, 512], F32, name="yp", tag="prj", bufs=2)
                for p in range(2):
                    nc.tensor.matmul(
                        yp,
                        woT[:, p, t * 128 : (t + 1) * 128],
                        xaT[:, p, c * 512 : (c + 1) * 512],
                        start=(p == 0),
                        stop=(p == 1),
                    )
                y_sb = yout.tile(
                    [128, 512], F16, name=f"y_sb{t}", tag=f"y{t % 4}"
                )
                if act_copy:
                    nc.scalar.copy(y_sb, yp)
                else:
                    nc.vector.tensor_copy(y_sb, yp)
                nc.sync.dma_start(
                    out=yT_d.ap()[
                        t * 128 : (t + 1) * 128, c * 512 : (c + 1) * 512
                    ],
                    in_=y_sb,
                )

            def pv_j(c, hp, j, op, pT):
                """Inline PV: one accumulation matmul per head for key-tile
                j, right after its exp. op = (op_h0, op_h1)."""
                jmax = 4 * c + 3
                off = max(0, (j - 4 * c) * 128)
                w = 512 - off
                for hr in range(2):
                    nc.tensor.matmul(
                        op[hr][:, off:512],
                        vp[j][:, 2 * hp + hr, :],
                        pT[:, hr * 512 : hr * 512 + w],
                        start=(j == 0),
                        stop=(j == jmax),
                    )

            pend = []

            def finish_unit():
                c, hp, op, pT_last = pend.pop()
                pv_j(c, hp, 4 * c + 3, op, pT_last)
                for hr in range(2):
                    rrow = norm.tile([1, 512], F32, name="rrow", tag="rrow")
                    nc.vector.reciprocal(rrow, op[hr][DV : DV + 1, :])
                    rrec = norm.tile([64, 512], F32, name="rrec", tag="rrec")
                    nc.gpsimd.partition_broadcast(rrec, rrow)
                    nc.vector.tensor_mul(
                        xaT[
                            hr * 64 : (hr + 1) * 64,
                            hp,
                            c * 512 : (c + 1) * 512,
                        ],
                        op[hr][0:DV, :],
                        rrec,
                    )

            def unit(c, hp, fillers, inline_pv=True):
                """Fused S+exp+PV burst for head pair hp of chunk c.

                Per j: two DoubleRow score matmuls, one wide bf16 exp, a
                GPSIMD causal-mask multiply on diagonal tiles, and the
                previous j's PV accumulation (PSUM groups stay open across
                the burst). Normalization closes the unit."""
                jmax = 4 * c + 3
                fi = list(fillers)
                op = [
                    ps.tile([DV + 1, 512], F32, name=f"op{hr}", tag="op",
                            bufs=2)
                    for hr in range(2)
                ]
                pts = []
                for j in range(jmax + 1):
                    off = max(0, (j - 4 * c) * 128)
                    w = 512 - off
                    sp = ps.tile([128, 1024], F32, name="sp", tag="sp", bufs=2)
                    pT = work.tile([128, 1024], BF, name="pT", tag="pT",
                                   bufs=8)
                    for hr in range(2):
                        kst = kT[hp][
                            hr * 64 : (hr + 1) * 64, j * 128 : (j + 1) * 128
                        ]
                        nc.tensor.matmul(
                            sp[:, hr * 512 : hr * 512 + w],
                            kst[:, None, :].broadcast_to([64, 2, 128]),
                            qT[hp][
                                hr * 64 : (hr + 1) * 64,
                                :,
                                c * 512 + off : (c + 1) * 512,
                            ],
                            start=True,
                            stop=True,
                            perf_mode=DR,
                        )
                    if j == 0 and pend:
                        finish_unit()
                    if inline_pv and j > 0:
                        pv_j(c, hp, j - 1, op, pts[j - 1])
                    if off:
                        spv = sp.rearrange("p (b k) -> p b k", b=2)[:, :, 0:w]
                        pTv = pT.rearrange("p (b k) -> p b k", b=2)[:, :, 0:w]
                        nc.scalar.activation(pTv, spv, EXP, scale=ESCALE)
                    else:
                        nc.scalar.activation(pT, sp, EXP, scale=ESCALE)
                    if j >= 4 * c:
                        mv = pT.rearrange("p (b k) -> p b k", b=2)[:, :, 0:128]
                        nc.gpsimd.tensor_mul(
                            mv, mv, utm[:, None, :].broadcast_to([128, 2, 128])
                        )
                    pts.append(pT)
                    if fi:
                        fi.pop(0)()
                if inline_pv:
                    pend.append((c, hp, op, pts[jmax]))
                else:
                    for j in range(jmax):
                        pv_j(c, hp, j, op, pts[j])
                    pend.append((c, hp, op, pts[jmax]))
                for f in fi:
                    f()

            def F(fn, *a):
                return lambda: fn(*a)

            xks = (xk8, xkr8)
            xqs = (xq8, xqr8)
            fillers = {
                (0, 0): [F(proj_v, m) for m in range(0, 4)],
                (0, 1): [
                    F(proj_qk, xks, kT, 1, 0),
                    F(proj_qk, xks, kT, 1, 1),
                    F(proj_qk, xqs, qT, 1, 0),
                    F(proj_qk, xqs, qT, 1, 1),
                ],
                (1, 0): [F(proj_v, m) for m in range(4, 8)]
                + [
                    F(proj_qk, xks, kT, 2, 0),
                    F(proj_qk, xks, kT, 2, 1),
                    F(proj_qk, xqs, qT, 2, 0),
                    F(proj_qk, xqs, qT, 2, 1),
                ],
                (1, 1): [
                    F(proj_qk, xks, kT, 3, 0),
                    F(proj_qk, xks, kT, 3, 1),
                    F(proj_qk, xqs, qT, 3, 0),
                    F(proj_qk, xqs, qT, 3, 1),
                ],
                (3, 0): [F(proj_v, m) for m in range(8, 16)]
                + [F(outproj_t, 0, t) for t in range(8)],
                (3, 1): [F(outproj_t, 1, t) for t in range(8)],
                (2, 0): [F(outproj_t, 3, t) for t in range(8)],
                (2, 1): [],
            }

            # prologue: chunk-0 projections; later chunks are fillers
            for p in range(2):
                proj_qk(xks, kT, 0, p)
            for p in range(2):
                proj_qk(xqs, qT, 0, p)

            unit(0, 0, fillers[(0, 0)], inline_pv=False)
            for cu in [(0, 1), (1, 0), (1, 1), (3, 0), (3, 1),
                       (2, 0), (2, 1)]:
                unit(*cu, fillers[cu])
            finish_unit()
            for t in range(DT):
                outproj_t(2, t, act_copy=bool(t % 2))
    nc.compile()
    return nc


def kernel(**inputs):
    inputs = {k: np.asarray(v) for k, v in inputs.items()}
    Q, K, V = inputs["Q"], inputs["K"], inputs["V"]
    wq, bq, wo, bo = inputs["wq"], inputs["bq"], inputs["wo"], inputs["bo"]

    def f8pair(x, scale=1.0):
        """fp8 value + fp8 residual of x.T * scale."""
        y = np.asarray(x, np.float32).T * scale
        y8 = np.clip(y, -240, 240).astype(NPF8)
        r8 = (y - y8.astype(np.float32)).astype(NPF8)
        return np.ascontiguousarray(y8), np.ascontiguousarray(r8)

    def bfT(x):
        return np.ascontiguousarray(np.asarray(x, np.float32).T.astype(NPBF))

    xq = [f8pair(Q[b]) for b in range(B)]
    xk = [f8pair(K[b]) for b in range(B)]
    xv = [f8pair(V[b]) for b in range(B)]
    wqp = [f8pair(wq[g * DHC : (g + 1) * DHC, :], SQ) for g in range(4)]
    woT = [bfT(wo[:, g * DHC : (g + 1) * DHC]) for g in range(4)]
    bqs = [
        np.ascontiguousarray(bq[g * DHC : (g + 1) * DHC], np.float32) * SQ
        for g in range(4)
    ]
    utm = np.triu(np.ones((128, 128), np.float32)).astype(NPBF)
    zz = np.zeros((128, N), NPF8)

    if "nc" not in _CACHE:
        _CACHE["nc"] = build_nc()
    nc = _CACHE["nc"]

    in_maps = []
    for core in range(8):
        b, g = divmod(core, 4)
        in_maps.append(
            {
                "xq8": xq[b][0],
                "xqr8": xq[b][1],
                "xk8": xk[b][0],
                "xkr8": xk[b][1],
                "xv8": xv[b][0],
                "xvr8": xv[b][1],
                "wq8": wqp[g][0],
                "wqr8": wqp[g][1],
                "woT": woT[g],
                "bq": bqs[g],
                "utm": utm,
                "zz": zz,
            }
        )
    import os

    trace = bool(int(os.environ.get("KERNEL_TRACE", "0")))
    try:
        res = run_bass_kernel_spmd(
            nc, in_maps, core_ids=list(range(8)), trace=trace
        )
    except ModuleNotFoundError:
        res = run_bass_kernel_spmd(nc, in_maps, core_ids=list(range(8)))
    _CACHE["last_results"] = res

    out = np.empty((B, N, D), np.float32)
    for b in range(B):
        acc = res.results[4 * b]["yT"].astype(np.float32)
        for g in range(1, 4):
            acc += res.results[4 * b + g]["yT"]
        out[b] = acc.T * (1.0 / OSCALE) + bo
    return out


# revision 16
# speedup vs baseline: 1.1180x; 1.1180x over previous
"""Multi-head attention (B=2, N=2048, d_model=1024, H=16) on 8 NeuronCores.

Sharding: data-parallel on batch (2) x tensor-parallel on heads (4 groups of
4 heads). Core c handles batch c//4, head-group c%4; the host sums the 4
output-projection partials per batch.

Precision plan (gate is relmax 2e-2; attention here averages ~uniformly over
~10^3 keys, so the output is CLT-small and independent per-key quantization
noise does NOT average away relative to it — plain fp8 anywhere in the
v->output chain costs ~2-3e-2 alone):
  - Projections run as residual-fp8: the host ships x and 32*wq as an fp8
    value plus an fp8 residual; three DoubleRow passes (x8*w8 + xr8*w8 +
    x8*wr8) give ~bf16 accuracy at 0.75x the bf16 matmul cost.
  - q/k are STORED fp8 (the one affordable lossy point, ~1.2e-2): score
    matmuls then run DoubleRow at 0.5 cycles/row by pairing the real K-tile
    with a zero slot (stride-0 {k,k} stationary x {q,0} moving).
  - P, V, attention output, and the output projection stay bf16.

Softmax skips max-subtraction (scores bounded ~+-3): exp runs on ACT only
(~73us), second to the PE (~88us bottleneck). DVE does the PSUM->SBUF moves
(bias adds, normalization, output copies); GPSIMD does the causal mask
multiplies and the denominator broadcasts.
"""

import sys

if "/opt/trn_rl_repo" not in sys.path:
    sys.path.insert(0, "/opt/trn_rl_repo")

import numpy as np
import ml_dtypes

import concourse.bass as bass
import concourse.mybir as mybir
import concourse.tile as tile
from concourse import bacc
from concourse.bass_utils import run_bass_kernel_spmd

B, N, D, H = 2, 2048, 1024, 16
DV = D // H  # 64
HPC = H // 4  # heads per core: 4
DHC = HPC * DV  # head dims per core: 256
DT = D // 128  # 8 din-tiles
F8 = mybir.dt.float8e4
BF = mybir.dt.bfloat16
F32 = mybir.dt.float32
F16 = mybir.dt.float16
EXP = mybir.ActivationFunctionType.Exp
DR = mybir.MatmulPerfMode.DoubleRow
NPF8 = ml_dtypes.float8_e4m3
NPBF = ml_dtypes.bfloat16
SQ = 32.0  # wq pre-scale (q/k/v live at 32x true value on chip)
ESCALE = 0.125 / (SQ * SQ)  # exp scale: undo q*k scale, apply 1/sqrt(dv)
OSCALE = SQ  # output partial leaves chip at 32x (host divides)

_CACHE = {}


def build_nc():
    nc = bacc.Bacc("TRN2", target_bir_lowering=False, debug=False)
    dins = {}
    for nm in ("xq8", "xqr8", "xk8", "xkr8", "xv8", "xvr8"):
        dins[nm] = nc.dram_tensor(nm, [D, N], F8, kind="ExternalInput")
    wq8_d = nc.dram_tensor("wq8", [D, DHC], F8, kind="ExternalInput")
    wqr8_d = nc.dram_tensor("wqr8", [D, DHC], F8, kind="ExternalInput")
    woT_d = nc.dram_tensor("woT", [DHC, D], BF, kind="ExternalInput")
    bq_d = nc.dram_tensor("bq", [DHC], F32, kind="ExternalInput")
    utm_d = nc.dram_tensor("utm", [128, 128], BF, kind="ExternalInput")
    zz_d = nc.dram_tensor("zz", [128, N], F8, kind="ExternalInput")
    yT_d = nc.dram_tensor("yT", [D, N], F16, kind="ExternalOutput")

    with tile.TileContext(nc) as tc:
        with (
            tc.tile_pool(name="consts", bufs=1) as consts,
            tc.tile_pool(name="xin", bufs=1) as xin,
            tc.tile_pool(name="prod", bufs=1) as prod,
            tc.tile_pool(name="work", bufs=3) as work,
            tc.tile_pool(name="norm", bufs=3) as norm,
            tc.tile_pool(name="yout", bufs=2) as yout,
            tc.tile_pool(name="ps", bufs=1, space="PSUM") as ps,
        ):
            # ---- weights + constants ----
            wq8 = consts.tile([128, DT, DHC], F8, name="wq8")
            nc.sync.dma_start(
                out=wq8, in_=wq8_d.ap().rearrange("(j p) c -> p j c", p=128)
            )
            wqr8 = consts.tile([128, DT, DHC], F8, name="wqr8")
            nc.sync.dma_start(
                out=wqr8, in_=wqr8_d.ap().rearrange("(j p) c -> p j c", p=128)
            )
            bq_pp = consts.tile([128, 2], F32, name="bq_pp")
            nc.sync.dma_start(
                out=bq_pp, in_=bq_d.ap().rearrange("(c p) -> p c", p=128)
            )
            bq_row = consts.tile([1, DHC], F32, name="bq_row")
            nc.sync.dma_start(
                out=bq_row, in_=bq_d.ap().rearrange("(a c) -> a c", a=1)
            )
            bq_bc = consts.tile([128, DHC], F32, name="bq_bc")
            nc.gpsimd.partition_broadcast(bq_bc, bq_row)
            utm = consts.tile([128, 128], BF, name="utm")
            nc.sync.dma_start(out=utm, in_=utm_d.ap())

            # q/k projection outputs (fp8); qT slot 1 is a persistent zero
            # pad for the score DoubleRow trick
            qT = [prod.tile([128, 2, N], F8, name=f"qT{p}") for p in range(2)]
            kT = [prod.tile([128, N], F8, name=f"kT{p}") for p in range(2)]
            for p in range(2):
                nc.sync.dma_start(out=qT[p][:, 1, :], in_=zz_d.ap())
            xaT = prod.tile([128, 2, N], BF, name="xaT")

            # ---- bulk inputs: k first (kT-proj fills the load window) ----
            xk8 = xin.tile([128, DT, N], F8, name="xk8")
            xkr8 = xin.tile([128, DT, N], F8, name="xkr8")
            xq8 = xin.tile([128, DT, N], F8, name="xq8")
            xqr8 = xin.tile([128, DT, N], F8, name="xqr8")
            xv8 = xin.tile([128, DT, N], F8, name="xv8")
            xvr8 = xin.tile([128, DT, N], F8, name="xvr8")

            def load_slice(t, d, n0, n1):
                nc.sync.dma_start(
                    out=t[:, :, n0:n1],
                    in_=d.ap()[:, n0:n1].rearrange("(j p) n -> p j n", p=128),
                )

            def load_j2(t, d, j2):
                nc.sync.dma_start(
                    out=t[:, j2 : j2 + 2, :],
                    in_=d.ap()[j2 * 128 : (j2 + 2) * 128, :].rearrange(
                        "(j p) n -> p j n", p=128
                    ),
                )

            # incremental chunk-wise loads matched to the unit schedule:
            # chunk c of k/q lands just before the (c,*) bursts need it
            def load_chunk(c, tensors):
                for t, d in tensors:
                    load_slice(t, d, c * 512, (c + 1) * 512)

            kq = [(xk8, dins["xk8"]), (xkr8, dins["xkr8"]),
                  (xq8, dins["xq8"]), (xqr8, dins["xqr8"])]
            vv = [(xv8, dins["xv8"]), (xvr8, dins["xvr8"])]
            load_chunk(0, kq)
            load_chunk(0, vv)
            load_chunk(1, kq)
            load_chunk(1, vv)
            load_chunk(2, kq)
            load_chunk(3, kq)
            load_chunk(2, vv)
            load_chunk(3, vv)
            woT = consts.tile([128, 2, D], BF, name="woT")
            nc.sync.dma_start(
                out=woT, in_=woT_d.ap().rearrange("(q p) c -> p q c", p=128)
            )

            # vp[m]: V tile for key-tile m, per head, with a trailing ones
            # column producing the softmax denominator (bf16)
            vp = [
                prod.tile([128, HPC, DV + 1], BF, name=f"vp{m}")
                for m in range(16)
            ]

            def proj_qk(xs, dst, c, p):
                """Residual-fp8 projection: 3 DoubleRow passes over 4
                din-tile pairs, accumulating in one PSUM group."""
                x8, xr8 = xs
                pp = ps.tile([128, 512], F32, name="prj_qk", tag="prj", bufs=2)
                passes = [(wq8, x8), (wq8, xr8), (wqr8, x8)]
                for pi, (w, x) in enumerate(passes):
                    for j2 in range(0, DT, 2):
                        nc.tensor.matmul(
                            pp,
                            w[:, j2 : j2 + 2, p * 128 : (p + 1) * 128],
                            x[:, j2 : j2 + 2, c * 512 : (c + 1) * 512],
                            start=(pi == 0 and j2 == 0),
                            stop=(pi == 2 and j2 == DT - 2),
                            perf_mode=DR,
                        )
                if dst is qT:
                    out = dst[p][:, 0, c * 512 : (c + 1) * 512]
                else:
                    out = dst[p][:, c * 512 : (c + 1) * 512]
                nc.vector.tensor_scalar_add(out, pp, bq_pp[:, p : p + 1])

            def proj_v(m):
                pv = ps.tile([128, 512], F32, name="prj_v", tag="prj", bufs=2)
                pvv = pv[:, 0:DHC]
                passes = [(xv8, wq8), (xvr8, wq8), (xv8, wqr8)]
                for pi, (x, w) in enumerate(passes):
                    for j2 in range(0, DT, 2):
                        nc.tensor.matmul(
                            pvv,
                            x[:, j2 : j2 + 2, m * 128 : (m + 1) * 128],
                            w[:, j2 : j2 + 2, :],
                            start=(pi == 0 and j2 == 0),
                            stop=(pi == 2 and j2 == DT - 2),
                            perf_mode=DR,
                        )
                nc.vector.tensor_add(
                    vp[m][:, :, 0:DV],
                    pvv.rearrange("p (h d) -> p h d", h=HPC),
                    bq_bc.rearrange("p (h d) -> p h d", h=HPC),
                )
                nc.vector.memset(vp[m][:, :, DV : DV + 1], 1.0)

            def outproj_t(c, t, act_copy=False):
                yp = ps.tile([128, 512], F32, name="yp", tag="prj", bufs=2)
                for p in range(2):
                    nc.tensor.matmul(
                        yp,
                        woT[:, p, t * 128 : (t + 1) * 128],
                        xaT[:, p, c * 512 : (c + 1) * 512],
                        start=(p == 0),
                        stop=(p == 1),
                    )
                y_sb = yout.tile(
                    [128, 512], F16, name=f"y_sb{t}", tag=f"y{t % 4}"
                )
                if act_copy:
                    nc.scalar.copy(y_sb, yp)
                else:
                    nc.vector.tensor_copy(y_sb, yp)
                nc.sync.dma_start(
                    out=yT_d.ap()[
                        t * 128 : (t + 1) * 128, c * 512 : (c + 1) * 512
                    ],
                    in_=y_sb,
                )

            def pv_j(c, hp, j, op, pT):
                """Inline PV: one accumulation matmul per head for key-tile
                j, right after its exp. op = (op_h0, op_h1)."""
                jmax = 4 * c + 3
                off = max(0, (j - 4 * c) * 128)
                w = 512 - off
                for hr in range(2):
                    nc.tensor.matmul(
                        op[hr][:, off:512],
                        vp[j][:, 2 * hp + hr, :],
                        pT[:, hr * 512 : hr * 512 + w],
                        start=(j == 0),
                        stop=(j == jmax),
                    )

            # pT tiles for unit u are consumed by PV in the next unit
            pT_tiles = {}

            def s_exp_burst(c, hp, fillers):
                """S+exp burst for head pair hp of chunk c."""
                jmax = 4 * c + 3
                fi = list(fillers)
                for j in range(jmax + 1):
                    off = max(0, (j - 4 * c) * 128)
                    w = 512 - off
                    sp = ps.tile([128, 1024], F32, name="sp", tag="sp", bufs=2)
                    pT = work.tile([128, 1024], BF, name="pT", tag="pT",
                                   bufs=26)
                    pT_tiles[(c, hp, j)] = pT
                    for hr in range(2):
                        kst = kT[hp][
                            hr * 64 : (hr + 1) * 64, j * 128 : (j + 1) * 128
                        ]
                        nc.tensor.matmul(
                            sp[:, hr * 512 : hr * 512 + w],
                            kst[:, None, :].broadcast_to([64, 2, 128]),
                            qT[hp][
                                hr * 64 : (hr + 1) * 64,
                                :,
                                c * 512 + off : (c + 1) * 512,
                            ],
                            start=True,
                            stop=True,
                            perf_mode=DR,
                        )
                    if off:
                        spv = sp.rearrange("p (b k) -> p b k", b=2)[:, :, 0:w]
                        pTv = pT.rearrange("p (b k) -> p b k", b=2)[:, :, 0:w]
                        nc.scalar.activation(pTv, spv, EXP, scale=ESCALE)
                    else:
                        nc.scalar.activation(pT, sp, EXP, scale=ESCALE)
                    if j >= 4 * c:
                        mv = pT.rearrange("p (b k) -> p b k", b=2)[:, :, 0:128]
                        nc.gpsimd.tensor_mul(
                            mv, mv, utm[:, None, :].broadcast_to([128, 2, 128])
                        )
                    if j % 2 and fi:
                        fi.pop(0)()
                for f in fi:
                    f()

            def pv_norm_pair(c, hp):
                jmax = 4 * c + 3
                for hr in range(2):
                    h = 2 * hp + hr
                    op = ps.tile(
                        [DV + 1, 512], F32, name="op", tag="op", bufs=2
                    )
                    for j in range(jmax + 1):
                        off = max(0, (j - 4 * c) * 128)
                        w = 512 - off
                        pT = pT_tiles[(c, hp, j)]
                        nc.tensor.matmul(
                            op[:, off:512],
                            vp[j][:, h, :],
                            pT[:, hr * 512 : hr * 512 + w],
                            start=(j == 0),
                            stop=(j == jmax),
                        )
                    if hr == 1:
                        for j in range(jmax + 1):
                            del pT_tiles[(c, hp, j)]
                    rrow = norm.tile([1, 512], F32, name="rrow", tag="rrow")
                    nc.vector.reciprocal(rrow, op[DV : DV + 1, :])
                    rrec = norm.tile([64, 512], F32, name="rrec", tag="rrec")
                    nc.gpsimd.partition_broadcast(rrec, rrow)
                    nc.vector.tensor_mul(
                        xaT[
                            hr * 64 : (hr + 1) * 64,
                            hp,
                            c * 512 : (c + 1) * 512,
                        ],
                        op[0:DV, :],
                        rrec,
                    )

            def F(fn, *a):
                return lambda: fn(*a)

            xks = (xk8, xkr8)
            xqs = (xq8, xqr8)
            fillers = {
                (0, 0): [],
                (0, 1): [F(proj_v, m) for m in range(0, 4)]
                + [
                    F(proj_qk, xks, kT, 1, 0),
                    F(proj_qk, xks, kT, 1, 1),
                    F(proj_qk, xqs, qT, 1, 0),
                    F(proj_qk, xqs, qT, 1, 1),
                ],
                (1, 0): [F(proj_v, m) for m in range(4, 8)]
                + [
                    F(proj_qk, xks, kT, 2, 0),
                    F(proj_qk, xks, kT, 2, 1),
                    F(proj_qk, xqs, qT, 2, 0),
                    F(proj_qk, xqs, qT, 2, 1),
                ],
                (1, 1): [
                    F(proj_qk, xks, kT, 3, 0),
                    F(proj_qk, xks, kT, 3, 1),
                    F(proj_qk, xqs, qT, 3, 0),
                    F(proj_qk, xqs, qT, 3, 1),
                ],
                (3, 0): [F(proj_v, m) for m in range(8, 12)]
                + [F(outproj_t, 0, t) for t in range(4)],
                (3, 1): [F(proj_v, m) for m in range(12, 16)]
                + [F(outproj_t, 0, t) for t in range(4, 8)]
                + [F(outproj_t, 1, t) for t in range(4)],
                (2, 0): [F(outproj_t, 1, t) for t in range(4, 8)],
                (2, 1): [F(outproj_t, 3, t) for t in range(8)],
            }

            # prologue: chunk-0 projections; later chunks are fillers
            for p in range(2):
                proj_qk(xks, kT, 0, p)
            for p in range(2):
                proj_qk(xqs, qT, 0, p)
            s_exp_burst(0, 0, fillers[(0, 0)])

            units = [(0, 1), (1, 0), (1, 1), (3, 0), (3, 1), (2, 0), (2, 1)]
            prev = (0, 0)
            for cu in units:
                s_exp_burst(*cu, fillers[cu])
                pv_norm_pair(*prev)
                prev = cu
            pv_norm_pair(*prev)
            for t in range(DT):
                outproj_t(2, t, act_copy=bool(t % 2))
    nc.compile()
    return nc


def kernel(**inputs):
    inputs = {k: np.asarray(v) for k, v in inputs.items()}
    Q, K, V = inputs["Q"], inputs["K"], inputs["V"]
    wq, bq, wo, bo = inputs["wq"], inputs["bq"], inputs["wo"], inputs["bo"]

    def f8pair(x, scale=1.0):
        """fp8 value + fp8 residual of x.T * scale."""
        y = np.asarray(x, np.float32).T * scale
        y8 = np.clip(y, -240, 240).astype(NPF8)
        r8 = (y - y8.astype(np.float32)).astype(NPF8)
        return np.ascontiguousarray(y8), np.ascontiguousarray(r8)

    def bfT(x):
        return np.ascontiguousarray(np.asarray(x, np.float32).T.astype(NPBF))

    xq = [f8pair(Q[b]) for b in range(B)]
    xk = [f8pair(K[b]) for b in range(B)]
    xv = [f8pair(V[b]) for b in range(B)]
    wqp = [f8pair(wq[g * DHC : (g + 1) * DHC, :], SQ) for g in range(4)]
    woT = [bfT(wo[:, g * DHC : (g + 1) * DHC]) for g in range(4)]
    bqs = [
        np.ascontiguousarray(bq[g * DHC : (g + 1) * DHC], np.float32) * SQ
        for g in range(4)
    ]
    utm = np.triu(np.ones((128, 128), np.float32)).astype(NPBF)
    zz = np.zeros((128, N), NPF8)

    if "nc" not in _CACHE:
        _CACHE["nc"] = build_nc()
    nc = _CACHE["nc"]

    in_maps = []
    for core in range(8):
        b, g = divmod(core, 4)
        in_maps.append(
            {
                "xq8": xq[b][0],
                "xqr8": xq[b][1],
                "xk8": xk[b][0],
                "xkr8": xk[b][1],
                "xv8": xv[b][0],
                "xvr8": xv[b][1],
                "wq8": wqp[g][0],
                "wqr8": wqp[g][1],
                "woT": woT[g],
                "bq": bqs[g],
                "utm": utm,
                "zz": zz,
            }
        )
    import os

    trace = bool(int(os.environ.get("KERNEL_TRACE", "0")))
    try:
        res = run_bass_kernel_spmd(
            nc, in_maps, core_ids=list(range(8)), trace=trace
        )
    except ModuleNotFoundError:
        res = run_bass_kernel_spmd(nc, in_maps, core_ids=list(range(8)))
    _CACHE["last_results"] = res

    out = np.empty((B, N, D), np.float32)
    for b in range(B):
        acc = res.results[4 * b]["yT"].astype(np.float32)
        for g in range(1, 4):
            acc += res.results[4 * b + g]["yT"]
        out[b] = acc.T * (1.0 / OSCALE) + bo
    return out
